# revision 10
# baseline (speedup 1.0000x reference)
"""Trainium2 Bass kernel for nn_ClusterModel (MoE routing + segment pooling).

Model:
  xg = x[group_indices]                         # [4, N/4, 128] per-group gather
  h  = relu(xg @ W1[g] + b1[g])                 # [4, N/4, 1024]
  og = h @ W2[g] + b2[g]                        # [4, N/4, 512]
  new_feat = scatter(og) back to node order     # [N, 512]
  emb = segment_max(new_feat, fine clusters)    # [8192, 512]  (16 nodes/cluster)
  normed = InstanceNorm per coarse graph        # [8192, 512]  (256 clusters/graph)
  logits = normed @ w_out + b_out               # [8192, 16]

Sharding: 8 cores, each takes N/8 = 16384 consecutive nodes = 1024 fine
clusters = 4 coarse graphs.  All segment reductions are core-local (cores
split exactly at coarse-graph boundaries) -> zero collectives.

v2 design (vs the f32r baseline):
  * bf16 everywhere on the data path (x, W1, W2, w_out, og scratch, emb).
    PSUM accumulation stays fp32.  b2 is dropped entirely: it is constant
    per channel, so max(og+b2) = max(og)+b2 and InstanceNorm's per-channel
    standardization cancels the shift exactly.
  * chunk-set-major main loop (row-chunk outer, group inner) so the og
    scatter-gather pipeline runs concurrently with the GEMMs: gather block
    t only needs og rows from chunk-sets <= ready_cs[t] (host-computed).
  * og rows go to DRAM in bf16; dma_gather(transpose=True) returns the
    (cluster, member) rows FEATURE-MAJOR, so the pairwise max tree directly
    yields emb in [feat, cluster] layout -- no PE transposes at all.
  * InstanceNorm stats reduce over each graph's real cluster range only
    (pad slots never pollute sums), then normalize + classifier run
    per-graph as soon as that graph's blocks are pooled (overlapped with
    remaining GEMM work).
"""

import numpy as np
from contextlib import ExitStack

import jax
import concourse.bass as bass
import concourse.tile as tile
from concourse import bacc, mybir
from concourse import bass2jax

F32 = mybir.dt.float32
BF16 = mybir.dt.bfloat16
I16 = mybir.dt.int16
AF = mybir.ActivationFunctionType
ALU = mybir.AluOpType

# Problem constants (hardcoded per contest contract)
N = 131072
D = 128
KEXP = 1024
H = 512
NG = 4
F_SEG = 8192
G_SEG = 32
C_CLS = 16
EPS = 1e-5
NCORES = 8
P = 128
NEG = -3.0e38

_PROGRAM_CACHE: dict = {}


def _chunk_sizes(gcap):
    """Row-chunk sizes per group for one capacity (multiples of 128)."""
    out = []
    r = gcap
    while r >= 512:
        out.append(512)
        r -= 512
    if r:
        out.append(r)
    return out


# ----------------------------------------------------------------------------
# Device program
# ----------------------------------------------------------------------------

def _build_program(GCAP: int, CCAP: int, MCAP: int, phases: int = 5,
                   repeat: int = 1, ready_cs: tuple = None):
    """Build the SPMD Bass program.

    GCAP: padded rows per (core, group), multiple of 128
    CCAP: padded clusters per (core, graph), multiple of 128
    MCAP: padded members per cluster, power of two
    ready_cs: per gather-block, index of the last chunk-set it needs
    phases: build only the first `phases` pipeline phases (debug bisection)
    repeat: wrap the whole body in a For_i loop (timing amortization)
    """
    CHUNKS = _chunk_sizes(GCAP)          # e.g. [512]*8 + [256]
    NCS = len(CHUNKS)
    OFFS = np.concatenate([[0], np.cumsum(CHUNKS)]).astype(int)
    RTOT = NG * GCAP                     # GEMM rows per core (padded)
    NROWS = 2 + RTOT                     # og scratch rows (0=zeros, 1=-inf)
    GPC = G_SEG // NCORES                # graphs per core = 4
    SLOTS = GPC * CCAP                   # cluster slots per core
    NBLK = SLOTS // P                    # gather blocks (128 clusters each)
    BPG = CCAP // P                      # blocks per graph
    KT = KEXP // P                       # 8 k-tiles in layer 2
    FT = H // P                          # 4 feature tiles of the 512-dim output
    NIDX = MCAP * P                      # gathered rows per block

    if ready_cs is None:
        ready_cs = tuple([NCS - 1] * NBLK)
    assert len(ready_cs) == NBLK
    # blocks to gather right after chunk-set cs completes
    blocks_at = {cs: [t for t in range(NBLK) if ready_cs[t] == cs]
                 for cs in range(NCS)}

    nc = bacc.Bacc("TRN2", target_bir_lowering=False, debug=False,
                   num_devices=NCORES)

    xt_ap = nc.dram_tensor("xt", [P, NG, GCAP], BF16, kind="ExternalInput").ap()
    w1_ap = nc.dram_tensor("w1", [P, NG, KEXP], BF16, kind="ExternalInput").ap()
    w2_ap = nc.dram_tensor("w2", [P, NG, KT, H], BF16, kind="ExternalInput").ap()
    b1_ap = nc.dram_tensor("b1s", [P, NG * KT], F32, kind="ExternalInput").ap()
    wo_ap = nc.dram_tensor("wout", [P, FT, C_CLS], BF16, kind="ExternalInput").ap()
    bo_ap = nc.dram_tensor("bout", [C_CLS, 1], F32, kind="ExternalInput").ap()
    ic_ap = nc.dram_tensor("invc", [P, GPC], F32, kind="ExternalInput").ap()
    gi_ap = nc.dram_tensor("gidx", [P, NBLK * (NIDX // 16)], I16,
                           kind="ExternalInput").ap()
    og_ap = nc.dram_tensor("ogs", [NROWS, H], BF16).ap()   # internal scratch
    lo_ap = nc.dram_tensor("logt", [C_CLS, SLOTS], F32, kind="ExternalOutput").ap()
    dbg_og_ap = dbg_emb_ap = None
    if phases <= 1:
        dbg_og_ap = nc.dram_tensor("dbg_og", [NROWS, H], BF16,
                                   kind="ExternalOutput").ap()
    elif phases <= 3:
        dbg_emb_ap = nc.dram_tensor("dbg_emb", [P, FT, SLOTS], BF16,
                                    kind="ExternalOutput").ap()

    # graph sizes in cluster slots are static (CCAP-padded); real sizes come
    # from the host via sz list captured in closure? -> sizes are data: the
    # reduce range must be static.  We reduce over the full CCAP range but
    # pad slots hold 0 (memset emb first), and mean uses the host-provided
    # 1/count, so sums are exact.
    with tile.TileContext(nc) as tc, ExitStack() as ctx:
        cst = ctx.enter_context(tc.tile_pool(name="cst", bufs=1))

        # --- resident constants -------------------------------------------
        w1_sb = cst.tile([P, NG, KEXP], BF16)
        nc.sync.dma_start(out=w1_sb[:], in_=w1_ap[:])
        b1_sb = cst.tile([P, NG * KT], F32)
        nc.sync.dma_start(out=b1_sb[:], in_=b1_ap[:])
        # preload chunk-set 0's activations ahead of the bulky W2 transfers so
        # the first GEMM1s only wait on W1 (+128KB of x)
        xt0_sb = [cst.tile([P, CHUNKS[0]], BF16, name=f"xt0_{g}")
                  for g in range(NG)]
        for g in range(NG):
            nc.sync.dma_start(out=xt0_sb[g][:], in_=xt_ap[:, g, 0:CHUNKS[0]])
        w2_sb = [cst.tile([P, KT, H], BF16, name=f"w2_{g}") for g in range(NG)]
        for g in range(NG):
            nc.sync.dma_start(out=w2_sb[g][:], in_=w2_ap[:, g, :, :])
        wo_sb = cst.tile([P, FT, C_CLS], BF16)
        nc.sync.dma_start(out=wo_sb[:], in_=wo_ap[:])
        bo_sb = cst.tile([C_CLS, 1], F32)
        nc.sync.dma_start(out=bo_sb[:], in_=bo_ap[:])
        ic_sb = cst.tile([P, GPC], F32)
        nc.sync.dma_start(out=ic_sb[:], in_=ic_ap[:])
        gi_sb = cst.tile([P, NBLK * (NIDX // 16)], I16)
        nc.sync.dma_start(out=gi_sb[:], in_=gi_ap[:])

        # og rows 0/1: zeros and -inf sentinels
        sent0 = cst.tile([1, H], BF16)
        nc.vector.memset(sent0[:], 0.0)
        nc.sync.dma_start(out=og_ap[0:1, :], in_=sent0[:])
        sent1 = cst.tile([1, H], BF16)
        nc.vector.memset(sent1[:], NEG)
        nc.sync.dma_start(out=og_ap[1:2, :], in_=sent1[:])

        emb_sb = cst.tile([P, FT, SLOTS], BF16)    # pooled embeddings, feat-major
        nc.vector.memset(emb_sb[:], 0.0)           # pad slots must read 0

        rep_cm = tc.For_i(0, repeat, 1) if repeat > 1 else None
        if rep_cm is not None:
            ctx.enter_context(rep_cm)

        # pools for the pipelined main body
        gxt = ctx.enter_context(tc.tile_pool(name="g_xt", bufs=3))
        ght = ctx.enter_context(tc.tile_pool(name="g_ht", bufs=12))
        gog = ctx.enter_context(tc.tile_pool(name="g_og", bufs=3))
        gph = ctx.enter_context(tc.tile_pool(name="g_ph", bufs=4, space="PSUM"))
        gpo = ctx.enter_context(tc.tile_pool(name="g_po", bufs=3, space="PSUM"))
        # pooling pools
        pga = ctx.enter_context(tc.tile_pool(name="p_gat", bufs=2))
        ptr = ctx.enter_context(tc.tile_pool(name="p_tree", bufs=2))
        # norm + classifier pools
        pnm = ctx.enter_context(tc.tile_pool(name="p_nrm", bufs=2))
        pcl = ctx.enter_context(tc.tile_pool(name="p_cls", bufs=2))
        pcp = ctx.enter_context(tc.tile_pool(name="p_cps", bufs=1, space="PSUM"))

        eng_flip = [0]

        def pool_block(t):
            """Gather block t (128 clusters x MCAP members) + max tree."""
            idx_sl = gi_sb[:, t * (NIDX // 16):(t + 1) * (NIDX // 16)]
            gat = pga.tile([P, FT, NIDX], BF16, tag="gat")
            nc.gpsimd.dma_gather(
                gat[:], og_ap[:], idx_sl, NIDX, NIDX, H,
                transpose=True, single_packet=False)
            cur = gat
            m = MCAP
            while m > 1:
                m //= 2
                half = m * P
                if m == 1:
                    nxt_ap = emb_sb[:, :, t * P:(t + 1) * P]
                    nc.vector.tensor_tensor(
                        out=nxt_ap, in0=cur[:, :, 0:half],
                        in1=cur[:, :, half:2 * half], op=ALU.max)
                else:
                    nxt = ptr.tile([P, FT, half], BF16, tag=f"tm{m}")
                    nc.vector.tensor_tensor(
                        out=nxt[:], in0=cur[:, :, 0:half],
                        in1=cur[:, :, half:2 * half], op=ALU.max)
                    cur = nxt

        def norm_and_classify(gi):
            """InstanceNorm + classifier for graph gi (CCAP cluster slots)."""
            slab = emb_sb[:, :, gi * CCAP:(gi + 1) * CCAP]
            sm = pnm.tile([P, FT], F32, tag="sm")
            nc.vector.tensor_reduce(sm[:], slab, mybir.AxisListType.X, ALU.add)
            sq = pnm.tile([P, FT, CCAP], F32, tag="sq")
            nc.scalar.activation(sq[:], slab, AF.Square)
            s2 = pnm.tile([P, FT], F32, tag="s2")
            nc.vector.tensor_reduce(s2[:], sq[:], mybir.AxisListType.X, ALU.add)
            mean = pnm.tile([P, FT], F32, tag="mean")
            nc.vector.tensor_scalar(mean[:], sm[:], ic_sb[:, gi:gi + 1], None,
                                    op0=ALU.mult)
            ex2 = pnm.tile([P, FT], F32, tag="ex2")
            nc.vector.tensor_scalar(ex2[:], s2[:], ic_sb[:, gi:gi + 1], None,
                                    op0=ALU.mult)
            var = pnm.tile([P, FT], F32, tag="var")
            # var = ex2 - mean^2 ; then rstd = 1/sqrt(var+eps)
            m2 = pnm.tile([P, FT], F32, tag="m2")
            nc.vector.tensor_tensor(out=m2[:], in0=mean[:], in1=mean[:],
                                    op=ALU.mult)
            nc.vector.tensor_tensor(out=var[:], in0=ex2[:], in1=m2[:],
                                    op=ALU.subtract)
            ve = pnm.tile([P, FT], F32, tag="ve")
            nc.vector.tensor_scalar_add(ve[:], var[:], EPS)
            sd = pnm.tile([P, FT], F32, tag="sd")
            nc.scalar.activation(sd[:], ve[:], AF.Sqrt)
            rstd = pnm.tile([P, FT], F32, tag="rstd")
            nc.vector.reciprocal(rstd[:], sd[:])
            embn = pcl.tile([P, FT, CCAP], BF16, tag="embn")
            for f in range(FT):
                nc.vector.tensor_scalar(
                    embn[:, f, :], emb_sb[:, f, gi * CCAP:(gi + 1) * CCAP],
                    mean[:, f:f + 1], rstd[:, f:f + 1],
                    op0=ALU.subtract, op1=ALU.mult)
            for n0 in range(0, CCAP, 512):
                nw = min(512, CCAP - n0)
                lg_ps = pcp.tile([C_CLS, 512], F32, tag="lg")
                for f in range(FT):
                    nc.tensor.matmul(lg_ps[:, :nw], wo_sb[:, f, :],
                                     embn[:, f, n0:n0 + nw],
                                     start=(f == 0), stop=(f == FT - 1))
                lg_sb = pcl.tile([C_CLS, 512], F32, tag="lgs")
                nc.vector.tensor_scalar(lg_sb[:, :nw], lg_ps[:, :nw],
                                        bo_sb[:], None, op0=ALU.add)
                nc.sync.dma_start(
                    out=lo_ap[:, gi * CCAP + n0:gi * CCAP + n0 + nw],
                    in_=lg_sb[:, :nw])

        pooled_blocks = [False] * NBLK

        # --- main pipelined loop ------------------------------------------
        for cs in range(NCS):
            cw = CHUNKS[cs]
            off = int(OFFS[cs])
            SB = cw // P                       # s-blocks in this chunk
            for g in range(NG):
                if cs == 0:
                    xt_sb = xt0_sb[g]
                else:
                    xt_sb = gxt.tile([P, 512], BF16, tag="xt")
                    nc.sync.dma_start(out=xt_sb[:, :cw],
                                      in_=xt_ap[:, g, off:off + cw])
                ht = []
                for kt in range(KT):
                    h_ps = gph.tile([P, 512], F32, tag="h")
                    nc.tensor.matmul(
                        h_ps[:, :cw], w1_sb[:, g, kt * P:(kt + 1) * P],
                        xt_sb[:, :cw], start=True, stop=True)
                    ht_sb = ght.tile([P, 512], BF16, tag="ht")
                    bcol = b1_sb[:, g * KT + kt:g * KT + kt + 1]
                    # all relus on ACT: keeps DVE free for the max trees so
                    # the PSUM-recycling chain (relu -> h_ps free) never
                    # queues behind a 4.4us tree op
                    nc.scalar.activation(ht_sb[:, :cw], h_ps[:, :cw],
                                         AF.Relu, bias=bcol)
                    ht.append(ht_sb)
                og_sb = gog.tile([P, SB, H], BF16, tag="og")
                for s in range(SB):
                    og_ps = gpo.tile([P, H], F32, tag="og")
                    for kt in range(KT):
                        nc.tensor.matmul(
                            og_ps[:], ht[kt][:, s * P:(s + 1) * P],
                            w2_sb[g][:, kt, :],
                            start=(kt == 0), stop=(kt == KT - 1))
                    if s != 3:
                        nc.scalar.activation(og_sb[:, s, :], og_ps[:], AF.Copy)
                    else:
                        nc.vector.tensor_copy(og_sb[:, s, :], og_ps[:])
                r0 = 2 + g * GCAP + off
                dst = og_ap[r0:r0 + cw, :].rearrange("(s p) h -> p s h", p=P)
                nc.sync.dma_start(out=dst, in_=og_sb[:, :SB, :])

            if phases >= 2:
                for t in blocks_at.get(cs, []):
                    pool_block(t)
                    pooled_blocks[t] = True
                    if phases >= 4:
                        gi = t // BPG
                        if all(pooled_blocks[gi * BPG:(gi + 1) * BPG]):
                            norm_and_classify(gi)

        if dbg_og_ap is not None:
            nc.sync.dma_start(out=dbg_og_ap[:], in_=og_ap[:])
        if dbg_emb_ap is not None:
            nc.sync.dma_start(out=dbg_emb_ap[:], in_=emb_sb[:])

    nc.compile()
    return nc


# ----------------------------------------------------------------------------
# PJRT runner (mirrors bass2jax.run_bass_via_pjrt, but reusable for timing)
# ----------------------------------------------------------------------------

class _Runner:
    def __init__(self, nc):
        from jax.sharding import Mesh, PartitionSpec
        from jax.experimental.shard_map import shard_map

        bass2jax.install_neuronx_cc_hook()
        self.nc = nc
        part_name = (nc.partition_id_tensor.name
                     if nc.partition_id_tensor else None)
        in_names, out_names, out_avals, zero_outs = [], [], [], []
        for alloc in nc.m.functions[0].allocations:
            if not isinstance(alloc, mybir.MemoryLocationSet):
                continue
            name = alloc.memorylocations[0].name
            if alloc.kind == "ExternalInput":
                if name != part_name:
                    in_names.append(name)
            elif alloc.kind == "ExternalOutput":
                out_names.append(name)
                shape = tuple(alloc.tensor_shape)
                dtype = mybir.dt.np(alloc.dtype)
                out_avals.append(jax.core.ShapedArray(shape, dtype))
                zero_outs.append(np.zeros(shape, dtype))
        self.n_params = len(in_names)
        self.in_names = in_names + out_names
        if part_name is not None:
            self.in_names = self.in_names + [part_name]
        self.out_names = out_names
        self.out_avals = out_avals
        self.zero_outs = zero_outs

        def _body(*args):
            operands = list(args)
            if part_name is not None:
                operands.append(bass2jax.partition_id_tensor())
            outs = bass2jax._bass_exec_p.bind(
                *operands,
                out_avals=tuple(out_avals),
                in_names=tuple(self.in_names),
                out_names=tuple(out_names),
                lowering_input_output_aliases=(),
                sim_require_finite=True,
                sim_require_nnan=True,
                nc=nc,
            )
            return tuple(outs)

        devices = jax.devices()[:NCORES]
        self.mesh = Mesh(np.asarray(devices), ("core",))
        n_all = self.n_params + len(out_names)
        self.fn = jax.jit(
            shard_map(_body, mesh=self.mesh,
                      in_specs=(PartitionSpec("core"),) * n_all,
                      out_specs=(PartitionSpec("core"),) * len(out_names),
                      check_rep=False),
            keep_unused=True,
        )

    def prepare(self, in_maps):
        concat = [
            np.concatenate([np.asarray(m[nm]) for m in in_maps], axis=0)
            for nm in self.in_names[:self.n_params]
        ]
        concat += [
            np.zeros((NCORES * z.shape[0], *z.shape[1:]), z.dtype)
            for z in self.zero_outs
        ]
        return concat

    def run(self, args):
        outs = self.fn(*args)
        return [
            {nm: np.asarray(outs[i]).reshape(NCORES, *self.out_avals[i].shape)[c]
             for i, nm in enumerate(self.out_names)}
            for c in range(NCORES)
        ]


# ----------------------------------------------------------------------------
# Host-side sharding / index plumbing
# ----------------------------------------------------------------------------

def _round_up(v, m):
    return (v + m - 1) // m * m


def _pow2_round(v):
    p = 1
    while p < v:
        p *= 2
    return p


def prepare(x, group_indices, pool_cluster_fine, batch_cluster_coarse,
            W1, b1, W2, b2, w_out, b_out):
    """Compute capacities + per-core input maps. Returns (key, in_maps, meta)."""
    bf16 = mybir.dt.np(BF16)
    x = np.asarray(x, dtype=np.float32)
    gidx = np.asarray(group_indices)
    pcf = np.asarray(pool_cluster_fine).astype(np.int64)
    bcc = np.asarray(batch_cluster_coarse).astype(np.int64)
    W1 = np.asarray(W1, dtype=np.float32)
    b1 = np.asarray(b1, dtype=np.float32)
    W2 = np.asarray(W2, dtype=np.float32)
    w_out = np.asarray(w_out, dtype=np.float32)
    b_out = np.asarray(b_out, dtype=np.float32)
    # b2 is provably irrelevant: per-channel constant shift before a
    # per-channel max and instance-norm standardization -> cancels exactly.

    GPC = G_SEG // NCORES

    # node -> group (later groups win on duplicates, matching scatter order)
    gid = np.full(N, -1, np.int32)
    for g in range(NG):
        gid[gidx[g]] = g

    # graph/cluster/node boundaries (general sorted-segment support)
    fine_lo = np.searchsorted(bcc, np.arange(0, G_SEG, GPC))          # per core
    fine_hi = np.searchsorted(bcc, np.arange(GPC - 1, G_SEG, GPC), "right")
    node_lo = np.searchsorted(pcf, fine_lo)
    node_hi = np.searchsorted(pcf, fine_hi)

    # cluster boundaries for every fine cluster
    cl_lo = np.searchsorted(pcf, np.arange(F_SEG))
    cl_hi = np.searchsorted(pcf, np.arange(F_SEG), "right")
    cl_sz = cl_hi - cl_lo
    MCAP = _pow2_round(max(1, int(cl_sz.max())))

    # graph boundaries in cluster space, per core
    g_lo = np.searchsorted(bcc, np.arange(G_SEG))
    g_hi = np.searchsorted(bcc, np.arange(G_SEG), "right")
    g_sz = g_hi - g_lo
    CCAP = _round_up(max(1, int(g_sz.max())), P)

    # rows per (core, group)
    counts = np.zeros((NCORES, NG), np.int64)
    core_nodes = []
    for c in range(NCORES):
        nd = np.arange(node_lo[c], node_hi[c])
        core_nodes.append(nd)
        gs = gid[nd]
        for g in range(NG):
            counts[c, g] = int((gs == g).sum())
    GCAP = _round_up(max(1, int(counts.max())), 256)
    RTOT = NG * GCAP
    assert 2 + RTOT < 32768, f"GCAP={GCAP} too large for int16 gather indices"
    SLOTS = GPC * CCAP
    NBLK = SLOTS // P
    NIDX = MCAP * P
    CHUNKS = _chunk_sizes(GCAP)
    OFFS = np.concatenate([[0], np.cumsum(CHUNKS)]).astype(int)

    # replicated weight prep (shared across cores)
    w1_h = np.ascontiguousarray(W1.transpose(1, 0, 2)).astype(bf16)
    w2_h = np.ascontiguousarray(
        W2.reshape(NG, KEXP // P, P, H).transpose(2, 0, 1, 3)).astype(bf16)
    b1_h = np.ascontiguousarray(
        b1.reshape(NG, KEXP // P, P).transpose(2, 0, 1).reshape(P, -1))
    wo_h = np.ascontiguousarray(
        w_out.reshape(H // P, P, C_CLS).transpose(1, 0, 2)).astype(bf16)
    bo_h = np.ascontiguousarray(b_out.reshape(C_CLS, 1))

    in_maps = []
    meta = []
    ready_all = np.zeros(NBLK, np.int64)
    for c in range(NCORES):
        nd = core_nodes[c]
        gs = gid[nd]
        xt = np.zeros((P, NG, GCAP), bf16)
        rows = np.zeros(N, np.int32)     # node -> og row (0 = zero row)
        for g in range(NG):
            sel = nd[gs == g]
            cnt = len(sel)
            xt[:, g, :cnt] = x[sel].T.astype(bf16)
            rows[sel] = 2 + g * GCAP + np.arange(cnt, dtype=np.int32)

        # member table: [SLOTS, MCAP] og-row indices.  Unused slots keep all
        # members = row 0 (zeros -> emb 0, sums unaffected); real clusters
        # pad their member tail with row 1 (-inf, neutral for max).
        member = np.zeros((SLOTS, MCAP), np.int32)
        clusters_c = np.arange(fine_lo[c], fine_hi[c])
        inv_cnt = np.zeros(GPC, np.float32)
        for gi in range(GPC):
            gg = c * GPC + gi
            inv_cnt[gi] = 1.0 / max(int(g_sz[gg]), 1)
        for f in clusters_c:
            gi = int(bcc[f]) - c * GPC
            j = int(f) - int(g_lo[int(bcc[f])])
            slot = gi * CCAP + j
            sz = int(cl_sz[f])
            member[slot, :sz] = rows[int(cl_lo[f]) + np.arange(sz)]
            member[slot, sz:] = 1

        # wrap indices for dma_gather: per block t, seq i = m*128 + cluster
        gidx_w = np.zeros((P, NBLK * (NIDX // 16)), np.int16)
        for t in range(NBLK):
            mt = member[t * P:(t + 1) * P]               # [128, MCAP]
            seq = mt.T.reshape(-1)                        # i = m*128 + c
            w = seq.reshape(-1, 16).T.astype(np.int16)    # [16, NIDX/16]
            gidx_w[:, t * (NIDX // 16):(t + 1) * (NIDX // 16)] = \
                np.tile(w, (8, 1))

        # gather-block readiness: last chunk-set containing any member row
        rank = (member - 2) % GCAP                        # rank within group
        rank[member < 2] = 0                              # sentinels: ready at 0
        cs_of_rank = np.searchsorted(OFFS[1:], rank, "right")
        ready = cs_of_rank.reshape(NBLK, P * MCAP).max(axis=1)
        ready_all = np.maximum(ready_all, ready)

        in_maps.append({
            "xt": xt,
            "w1": w1_h, "w2": w2_h, "b1s": b1_h,
            "wout": wo_h, "bout": bo_h,
            "invc": np.broadcast_to(inv_cnt[None, :], (P, GPC)).copy(),
            "gidx": gidx_w,
        })
        meta.append({"clusters": clusters_c, "fine_lo": int(fine_lo[c]),
                     "g_lo": g_lo, "c": c})

    key = (GCAP, CCAP, MCAP, tuple(int(v) for v in ready_all))
    return key, in_maps, meta, (CCAP,)


def get_runner(key, phases=5, repeat=1):
    ck = (key, phases, repeat)
    if ck not in _PROGRAM_CACHE:
        GCAP, CCAP, MCAP, ready_cs = key
        nc = _build_program(GCAP, CCAP, MCAP, phases=phases, repeat=repeat,
                            ready_cs=ready_cs)
        _PROGRAM_CACHE[ck] = _Runner(nc)
    return _PROGRAM_CACHE[ck]


def kernel(**inputs) -> np.ndarray:
    key, in_maps, meta, (CCAP,) = prepare(**inputs)
    runner = get_runner(key)
    args = runner.prepare(in_maps)
    results = runner.run(args)

    bcc = np.asarray(inputs["batch_cluster_coarse"]).astype(np.int64)
    GPC = G_SEG // NCORES
    g_lo = np.searchsorted(bcc, np.arange(G_SEG))
    out = np.zeros((F_SEG, C_CLS), np.float32)
    for c in range(NCORES):
        lo = results[c]["logt"]              # [16, SLOTS]
        for f in meta[c]["clusters"]:
            gi = int(bcc[f]) - c * GPC
            slot = gi * CCAP + (int(f) - int(g_lo[int(bcc[f])]))
            out[f] = lo[:, slot]
    return out


# revision 35
# speedup vs baseline: 1.1433x; 1.1433x over previous
"""Trainium2 Bass kernel for nn_ClusterModel (MoE routing + segment pooling).

Model:
  xg = x[group_indices]                         # [4, N/4, 128] per-group gather
  h  = relu(xg @ W1[g] + b1[g])                 # [4, N/4, 1024]
  og = h @ W2[g] + b2[g]                        # [4, N/4, 512]
  new_feat = scatter(og) back to node order     # [N, 512]
  emb = segment_max(new_feat, fine clusters)    # [8192, 512]  (16 nodes/cluster)
  normed = InstanceNorm per coarse graph        # [8192, 512]  (256 clusters/graph)
  logits = normed @ w_out + b_out               # [8192, 16]

Sharding: 8 cores, each takes N/8 = 16384 consecutive nodes = 1024 fine
clusters = 4 coarse graphs.  All segment reductions are core-local (cores
split exactly at coarse-graph boundaries) -> zero collectives.

v2 design (vs the f32r baseline):
  * bf16 everywhere on the data path (x, W1, W2, w_out, og scratch, emb).
    PSUM accumulation stays fp32.  b2 is dropped entirely: it is constant
    per channel, so max(og+b2) = max(og)+b2 and InstanceNorm's per-channel
    standardization cancels the shift exactly.
  * chunk-set-major main loop (row-chunk outer, group inner) so the og
    scatter-gather pipeline runs concurrently with the GEMMs: gather block
    t only needs og rows from chunk-sets <= ready_cs[t] (host-computed).
  * og rows go to DRAM in bf16; dma_gather(transpose=True) returns the
    (cluster, member) rows FEATURE-MAJOR, so the pairwise max tree directly
    yields emb in [feat, cluster] layout -- no PE transposes at all.
  * InstanceNorm stats reduce over each graph's real cluster range only
    (pad slots never pollute sums), then normalize + classifier run
    per-graph as soon as that graph's blocks are pooled (overlapped with
    remaining GEMM work).
"""

import numpy as np
from contextlib import ExitStack

import jax
import concourse.bass as bass
import concourse.tile as tile
from concourse import bacc, mybir
from concourse import bass2jax

F32 = mybir.dt.float32
BF16 = mybir.dt.bfloat16
I16 = mybir.dt.int16
AF = mybir.ActivationFunctionType
ALU = mybir.AluOpType

# Problem constants (hardcoded per contest contract)
N = 131072
D = 128
KEXP = 1024
H = 512
NG = 4
F_SEG = 8192
G_SEG = 32
C_CLS = 16
EPS = 1e-5
NCORES = 8
P = 128
NEG = -3.0e38

_PROGRAM_CACHE: dict = {}


def _chunk_sizes(gcap):
    """Row-chunk sizes per group for one capacity (multiples of 128).

    The tail tapers (256s then 128s) so gather-block readiness advances in
    fine steps near the end -- late pooling then overlaps the remaining
    GEMM work instead of spilling past the final chunk."""
    out = []
    r = gcap
    while r >= 1024:
        out.append(512)
        r -= 512
    while r >= 384:
        out.append(256)
        r -= 256
    while r > 0:
        out.append(min(128, r))
        r -= 128
    return out


# ----------------------------------------------------------------------------
# Device program
# ----------------------------------------------------------------------------

def _build_program(GCAP: int, CCAP: int, MCAP: int, plan: tuple,
                   phases: int = 5, repeat: int = 1):
    """Build the SPMD Bass program.

    GCAP: padded rows per (core, group), multiple of 128
    CCAP: padded clusters per (core, graph), multiple of 128
    MCAP: padded members per cluster, power of two
    plan: tuple of (c0, ncl, ready_cs) gather-block descriptors
    phases: build only the first `phases` pipeline phases (debug bisection)
    repeat: wrap the whole body in a For_i loop (timing amortization)
    """
    CHUNKS = _chunk_sizes(GCAP)          # e.g. [512]*8 + [256]
    NCS = len(CHUNKS)
    OFFS = np.concatenate([[0], np.cumsum(CHUNKS)]).astype(int)
    RTOT = NG * GCAP                     # GEMM rows per core (padded)
    NROWS = 2 + RTOT                     # og scratch rows (0=zeros, 1=-inf)
    GPC = G_SEG // NCORES                # graphs per core = 4
    SLOTS = GPC * CCAP                   # cluster slots per core
    KT = KEXP // P                       # 8 k-tiles in layer 2
    FT = H // P                          # 4 feature tiles of the 512-dim output
    GI_COLS = sum(MCAP * ncl // 16 for (c0, ncl, rdy) in plan)
    items_at = {cs: [i for i, (c0, ncl, rdy) in enumerate(plan) if rdy == cs]
                for cs in range(NCS)}

    PRIO_POOL = 400        # pooling/norm ops yield to the GEMM-side stream

    nc = bacc.Bacc("TRN2", target_bir_lowering=False, debug=False,
                   num_devices=NCORES)

    xt_ap = nc.dram_tensor("xt", [P, NG, GCAP], BF16, kind="ExternalInput").ap()
    w1_ap = nc.dram_tensor("w1", [P, NG, KEXP], BF16, kind="ExternalInput").ap()
    w2_ap = nc.dram_tensor("w2", [P, NG, KT, H], BF16, kind="ExternalInput").ap()
    b1_ap = nc.dram_tensor("b1s", [P, NG * KT], F32, kind="ExternalInput").ap()
    wo_ap = nc.dram_tensor("wout", [P, FT, C_CLS], BF16, kind="ExternalInput").ap()
    bo_ap = nc.dram_tensor("bout", [C_CLS, 1], F32, kind="ExternalInput").ap()
    ic_ap = nc.dram_tensor("invc", [P, GPC], F32, kind="ExternalInput").ap()
    gi_ap = nc.dram_tensor("gidx", [P, GI_COLS], I16,
                           kind="ExternalInput").ap()
    og_ap = nc.dram_tensor("ogs", [NROWS, H], BF16).ap()   # internal scratch
    lo_ap = nc.dram_tensor("logt", [C_CLS, SLOTS], F32, kind="ExternalOutput").ap()
    dbg_og_ap = dbg_emb_ap = None
    if phases <= 1:
        dbg_og_ap = nc.dram_tensor("dbg_og", [NROWS, H], BF16,
                                   kind="ExternalOutput").ap()
    elif phases <= 3:
        dbg_emb_ap = nc.dram_tensor("dbg_emb", [P, FT, SLOTS], BF16,
                                    kind="ExternalOutput").ap()

    # graph sizes in cluster slots are static (CCAP-padded); real sizes come
    # from the host via sz list captured in closure? -> sizes are data: the
    # reduce range must be static.  We reduce over the full CCAP range but
    # pad slots hold 0 (memset emb first), and mean uses the host-provided
    # 1/count, so sums are exact.
    with tile.TileContext(nc) as tc, ExitStack() as ctx:
        cst = ctx.enter_context(tc.tile_pool(name="cst", bufs=1))

        # --- resident constants -------------------------------------------
        # startup order: (w1[g], xt0[g]) pairs so group 0's GEMM1 can begin
        # after ~384KB of DMA instead of the full weight set
        w1_sb = cst.tile([P, NG, KEXP], BF16)
        xt0_sb = [cst.tile([P, CHUNKS[0]], BF16, name=f"xt0_{g}")
                  for g in range(NG)]
        w2_sb = [cst.tile([P, KT, H], BF16, name=f"w2_{g}") for g in range(NG)]
        nc.sync.dma_start(out=w1_sb[:, 0, :], in_=w1_ap[:, 0, :])
        nc.sync.dma_start(out=xt0_sb[0][:], in_=xt_ap[:, 0, 0:CHUNKS[0]])
        b1_sb = cst.tile([P, NG * KT], F32)
        nc.sync.dma_start(out=b1_sb[:], in_=b1_ap[:])
        nc.sync.dma_start(out=w2_sb[0][:, 0:KT // 2, :],
                          in_=w2_ap[:, 0, 0:KT // 2, :])
        nc.sync.dma_start(out=w2_sb[0][:, KT // 2:, :],
                          in_=w2_ap[:, 0, KT // 2:, :])
        for g in range(1, NG):
            nc.sync.dma_start(out=w1_sb[:, g, :], in_=w1_ap[:, g, :])
            nc.sync.dma_start(out=xt0_sb[g][:], in_=xt_ap[:, g, 0:CHUNKS[0]])
            nc.sync.dma_start(out=w2_sb[g][:], in_=w2_ap[:, g, :, :])
        wo_sb = cst.tile([P, FT, C_CLS], BF16)
        nc.sync.dma_start(out=wo_sb[:], in_=wo_ap[:])
        bo_sb = cst.tile([C_CLS, 1], F32)
        nc.sync.dma_start(out=bo_sb[:], in_=bo_ap[:])
        ic_sb = cst.tile([P, GPC], F32)
        nc.sync.dma_start(out=ic_sb[:], in_=ic_ap[:])
        gi_sb = cst.tile([P, GI_COLS], I16)
        nc.sync.dma_start(out=gi_sb[:], in_=gi_ap[:])

        # og rows 0/1: zeros and -inf sentinels
        sent0 = cst.tile([1, H], BF16)
        nc.vector.memset(sent0[:], 0.0)
        nc.sync.dma_start(out=og_ap[0:1, :], in_=sent0[:])
        sent1 = cst.tile([1, H], BF16)
        nc.vector.memset(sent1[:], NEG)
        nc.sync.dma_start(out=og_ap[1:2, :], in_=sent1[:])

        emb_sb = cst.tile([P, FT, SLOTS], BF16)    # pooled embeddings, feat-major
        nc.vector.memset(emb_sb[:], 0.0)           # pad slots must read 0

        rep_cm = tc.For_i(0, repeat, 1) if repeat > 1 else None
        if rep_cm is not None:
            ctx.enter_context(rep_cm)

        # pools for the pipelined main body
        gxt = ctx.enter_context(tc.tile_pool(name="g_xt", bufs=8))
        ght = ctx.enter_context(tc.tile_pool(name="g_ht", bufs=12))
        gog = ctx.enter_context(tc.tile_pool(name="g_og", bufs=3))
        gph = ctx.enter_context(tc.tile_pool(name="g_ph", bufs=4, space="PSUM"))
        gpo = ctx.enter_context(tc.tile_pool(name="g_po", bufs=3, space="PSUM"))
        # pooling pools
        pga = ctx.enter_context(tc.tile_pool(name="p_gat", bufs=2))
        ptr = ctx.enter_context(tc.tile_pool(name="p_tree", bufs=2))
        # norm + classifier pools
        pnm = ctx.enter_context(tc.tile_pool(name="p_nrm", bufs=2))
        pcl = ctx.enter_context(tc.tile_pool(name="p_cls", bufs=2))
        pcp = ctx.enter_context(tc.tile_pool(name="p_cps", bufs=1, space="PSUM"))

        # per-graph stat accumulators (sum, sum of squares)
        acc_s = cst.tile([P, GPC, FT], F32)
        acc_q = cst.tile([P, GPC, FT], F32)

        item_off = []                   # idx-table column offset per plan item
        _o = 0
        for (c0, ncl, rdy) in plan:
            item_off.append(_o)
            _o += MCAP * ncl // 16

        def pool_steps(it):
            """Return the list of issue-steps (closures) for plan item it:
            gather -> max-tree levels (split per feature-block when wide) ->
            partial InstanceNorm stats."""
            c0, ncl, rdy = plan[it]
            nidx = MCAP * ncl
            steps = []
            state = {}

            def do_gather():
                idx_sl = gi_sb[:, item_off[it]:item_off[it] + nidx // 16]
                gat = pga.tile([P, FT, nidx], BF16, tag="gat")
                nc.gpsimd.dma_gather(
                    gat[:], og_ap[:], idx_sl, nidx, nidx, H,
                    transpose=True, single_packet=False)
                state["cur"] = gat
            steps.append(do_gather)

            m = MCAP
            while m > 1:
                m //= 2
                half = m * ncl

                def do_level(m=m, half=half):
                    with tc.high_priority(-PRIO_POOL):
                        cur = state["cur"]
                        if m == 1:
                            nxt_ap = emb_sb[:, :, c0:c0 + ncl]
                            nc.vector.tensor_tensor(
                                out=nxt_ap, in0=cur[:, :, 0:half],
                                in1=cur[:, :, half:2 * half], op=ALU.max)
                        else:
                            nxt = ptr.tile([P, FT, half], BF16, tag=f"tm{m}")
                            if half >= 1024:
                                for f in range(FT):
                                    nc.vector.tensor_tensor(
                                        out=nxt[:, f, :],
                                        in0=cur[:, f, 0:half],
                                        in1=cur[:, f, half:2 * half],
                                        op=ALU.max)
                            else:
                                nc.vector.tensor_tensor(
                                    out=nxt[:], in0=cur[:, :, 0:half],
                                    in1=cur[:, :, half:2 * half], op=ALU.max)
                            state["cur"] = nxt
                steps.append(do_level)

            if phases >= 4:
                def do_stats():
                  with tc.high_priority(-PRIO_POOL):
                    gi = c0 // CCAP
                    first = (c0 % CCAP == 0)
                    slab = emb_sb[:, :, c0:c0 + ncl]
                    sq = pnm.tile([P, FT, ncl], F32, tag="sq")
                    nc.scalar.activation(sq[:], slab, AF.Square)
                    if first:
                        nc.vector.tensor_reduce(acc_s[:, gi, :], slab,
                                                mybir.AxisListType.X, ALU.add)
                        nc.vector.tensor_reduce(acc_q[:, gi, :], sq[:],
                                                mybir.AxisListType.X, ALU.add)
                    else:
                        bs = pnm.tile([P, FT], F32, tag="bs")
                        nc.vector.tensor_reduce(bs[:], slab,
                                                mybir.AxisListType.X, ALU.add)
                        nc.vector.tensor_tensor(out=acc_s[:, gi, :],
                                                in0=acc_s[:, gi, :],
                                                in1=bs[:], op=ALU.add)
                        bq = pnm.tile([P, FT], F32, tag="bq")
                        nc.vector.tensor_reduce(bq[:], sq[:],
                                                mybir.AxisListType.X, ALU.add)
                        nc.vector.tensor_tensor(out=acc_q[:, gi, :],
                                                in0=acc_q[:, gi, :],
                                                in1=bq[:], op=ALU.add)
                steps.append(do_stats)
            return steps

        def norm_and_classify(gi):
          # InstanceNorm + classifier for graph gi (CCAP cluster slots)
          with tc.high_priority(-PRIO_POOL):
            mean = pnm.tile([P, FT], F32, tag="mean")
            nc.vector.tensor_scalar(mean[:], acc_s[:, gi, :],
                                    ic_sb[:, gi:gi + 1], None, op0=ALU.mult)
            ex2 = pnm.tile([P, FT], F32, tag="ex2")
            nc.vector.tensor_scalar(ex2[:], acc_q[:, gi, :],
                                    ic_sb[:, gi:gi + 1], None, op0=ALU.mult)
            var = pnm.tile([P, FT], F32, tag="var")
            # var = ex2 - mean^2 ; then rstd = 1/sqrt(var+eps)
            m2 = pnm.tile([P, FT], F32, tag="m2")
            nc.vector.tensor_tensor(out=m2[:], in0=mean[:], in1=mean[:],
                                    op=ALU.mult)
            nc.vector.tensor_tensor(out=var[:], in0=ex2[:], in1=m2[:],
                                    op=ALU.subtract)
            ve = pnm.tile([P, FT], F32, tag="ve")
            nc.vector.tensor_scalar_add(ve[:], var[:], EPS)
            sd = pnm.tile([P, FT], F32, tag="sd")
            nc.scalar.activation(sd[:], ve[:], AF.Sqrt)
            rstd = pnm.tile([P, FT], F32, tag="rstd")
            nc.vector.reciprocal(rstd[:], sd[:])
            embn = pcl.tile([P, FT, CCAP], BF16, tag="embn")
            for f in range(FT):
                nc.vector.tensor_scalar(
                    embn[:, f, :], emb_sb[:, f, gi * CCAP:(gi + 1) * CCAP],
                    mean[:, f:f + 1], rstd[:, f:f + 1],
                    op0=ALU.subtract, op1=ALU.mult)
            for n0 in range(0, CCAP, 512):
                nw = min(512, CCAP - n0)
                lg_ps = pcp.tile([C_CLS, 512], F32, tag="lg")
                for f in range(FT):
                    nc.tensor.matmul(lg_ps[:, :nw], wo_sb[:, f, :],
                                     embn[:, f, n0:n0 + nw],
                                     start=(f == 0), stop=(f == FT - 1))
                lg_sb = pcl.tile([C_CLS, 512], F32, tag="lgs")
                nc.vector.tensor_scalar(lg_sb[:, :nw], lg_ps[:, :nw],
                                        bo_sb[:], None, op0=ALU.add)
                nc.sync.dma_start(
                    out=lo_ap[:, gi * CCAP + n0:gi * CCAP + n0 + nw],
                    in_=lg_sb[:, :nw])

        # step queue: pool/norm work interleaved between GEMM group bodies
        graph_done = [0] * GPC
        step_q: list = []

        def enqueue_item(it):
            step_q.extend(pool_steps(it))
            c0, ncl, rdy = plan[it]
            gi = c0 // CCAP
            graph_done[gi] += ncl
            if phases >= 4 and graph_done[gi] == CCAP:
                step_q.append(lambda gi=gi: norm_and_classify(gi))

        def issue_steps(k):
            while k > 0 and step_q:
                step_q.pop(0)()
                k -= 1

        # --- main pipelined loop ------------------------------------------
        # GEMM2 runs one group behind GEMM1: between a group's first and
        # second half of GEMM1 k-tiles, PE executes the previous group's
        # GEMM2 -- every relu then has ~1.7us of slack before its h_ps slot
        # is needed again, so stray pool ops on ACT/DVE can't stall PE.
        prev = None                      # (ht, g, off, cw, SB, og_sb)

        def mm2_half(pv, half):
            ht_p, g_p, off_p, cw_p, SB_p, og_sb_p = pv
            s_lo = 0 if half == 0 else (SB_p + 1) // 2
            s_hi = (SB_p + 1) // 2 if half == 0 else SB_p
            for s in range(s_lo, s_hi):
                og_ps = gpo.tile([P, H], F32, tag="og")
                for kt in range(KT):
                    nc.tensor.matmul(
                        og_ps[:], ht_p[kt][:, s * P:(s + 1) * P],
                        w2_sb[g_p][:, kt, :],
                        start=(kt == 0), stop=(kt == KT - 1))
                if s % 2 == 0:
                    nc.scalar.activation(og_sb_p[:, s, :], og_ps[:], AF.Copy)
                else:
                    nc.vector.tensor_copy(og_sb_p[:, s, :], og_ps[:])
            if half == 1:
                r0 = 2 + g_p * GCAP + off_p
                dst = og_ap[r0:r0 + cw_p, :].rearrange("(s p) h -> p s h", p=P)
                nc.sync.dma_start(out=dst, in_=og_sb_p[:, :SB_p, :])

        xt_cur = list(xt0_sb)
        for cs in range(NCS):
            cw = CHUNKS[cs]
            off = int(OFFS[cs])
            xt_nxt: list = []
            SB = cw // P                       # s-blocks in this chunk
            if phases >= 2 and cs > 0:
                for it in items_at.get(cs - 1, []):
                    enqueue_item(it)
            for g in range(NG):
                xt_sb = xt_cur[g]
                # prefetch next chunk-set's activations (SP ring, ahead of
                # og writes which ride the Pool SWDGE ring)
                if cs + 1 < NCS:
                    nxt = gxt.tile([P, 512], BF16, tag="xt")
                    nc.scalar.dma_start(
                        out=nxt[:, :CHUNKS[cs + 1]],
                        in_=xt_ap[:, g, int(OFFS[cs + 1]):
                                  int(OFFS[cs + 1]) + CHUNKS[cs + 1]])
                    xt_nxt.append(nxt)
                ht = []

                def mm1_half(rng):
                    for kt in rng:
                        h_ps = gph.tile([P, 512], F32, tag="h")
                        nc.tensor.matmul(
                            h_ps[:, :cw], w1_sb[:, g, kt * P:(kt + 1) * P],
                            xt_sb[:, :cw], start=True, stop=True)
                        ht_sb = ght.tile([P, 512], BF16, tag="ht")
                        bcol = b1_sb[:, g * KT + kt:g * KT + kt + 1]
                        if kt % 2 == 0:
                            nc.scalar.activation(ht_sb[:, :cw], h_ps[:, :cw],
                                                 AF.Relu, bias=bcol)
                        else:
                            nc.vector.tensor_scalar(ht_sb[:, :cw],
                                                    h_ps[:, :cw], bcol, 0.0,
                                                    op0=ALU.add, op1=ALU.max)
                        ht.append(ht_sb)

                og_sb = gog.tile([P, SB, H], BF16, tag="og")
                mm1_half(range(0, KT // 2))
                if prev is not None:
                    mm2_half(prev, 0)
                mm1_half(range(KT // 2, KT))
                if prev is not None:
                    mm2_half(prev, 1)
                prev = (ht, g, off, cw, SB, og_sb)
                # interleave a few pooling/norm steps between group bodies
                issue_steps(max(1, cw // 128))
            issue_steps(len(step_q))
            xt_cur = xt_nxt
        if prev is not None:
            mm2_half(prev, 0)
            mm2_half(prev, 1)

        if phases >= 2:
            for it in items_at.get(NCS - 1, []):
                enqueue_item(it)
            issue_steps(len(step_q))

        if dbg_og_ap is not None:
            nc.sync.dma_start(out=dbg_og_ap[:], in_=og_ap[:])
        if dbg_emb_ap is not None:
            nc.sync.dma_start(out=dbg_emb_ap[:], in_=emb_sb[:])

    nc.compile()
    return nc


# ----------------------------------------------------------------------------
# PJRT runner (mirrors bass2jax.run_bass_via_pjrt, but reusable for timing)
# ----------------------------------------------------------------------------

class _Runner:
    def __init__(self, nc):
        from jax.sharding import Mesh, PartitionSpec
        from jax.experimental.shard_map import shard_map

        bass2jax.install_neuronx_cc_hook()
        self.nc = nc
        part_name = (nc.partition_id_tensor.name
                     if nc.partition_id_tensor else None)
        in_names, out_names, out_avals, zero_outs = [], [], [], []
        for alloc in nc.m.functions[0].allocations:
            if not isinstance(alloc, mybir.MemoryLocationSet):
                continue
            name = alloc.memorylocations[0].name
            if alloc.kind == "ExternalInput":
                if name != part_name:
                    in_names.append(name)
            elif alloc.kind == "ExternalOutput":
                out_names.append(name)
                shape = tuple(alloc.tensor_shape)
                dtype = mybir.dt.np(alloc.dtype)
                out_avals.append(jax.core.ShapedArray(shape, dtype))
                zero_outs.append(np.zeros(shape, dtype))
        self.n_params = len(in_names)
        self.in_names = in_names + out_names
        if part_name is not None:
            self.in_names = self.in_names + [part_name]
        self.out_names = out_names
        self.out_avals = out_avals
        self.zero_outs = zero_outs

        def _body(*args):
            operands = list(args)
            if part_name is not None:
                operands.append(bass2jax.partition_id_tensor())
            outs = bass2jax._bass_exec_p.bind(
                *operands,
                out_avals=tuple(out_avals),
                in_names=tuple(self.in_names),
                out_names=tuple(out_names),
                lowering_input_output_aliases=(),
                sim_require_finite=True,
                sim_require_nnan=True,
                nc=nc,
            )
            return tuple(outs)

        devices = jax.devices()[:NCORES]
        self.mesh = Mesh(np.asarray(devices), ("core",))
        n_all = self.n_params + len(out_names)
        self.fn = jax.jit(
            shard_map(_body, mesh=self.mesh,
                      in_specs=(PartitionSpec("core"),) * n_all,
                      out_specs=(PartitionSpec("core"),) * len(out_names),
                      check_rep=False),
            keep_unused=True,
        )

    def prepare(self, in_maps):
        concat = [
            np.concatenate([np.asarray(m[nm]) for m in in_maps], axis=0)
            for nm in self.in_names[:self.n_params]
        ]
        concat += [
            np.zeros((NCORES * z.shape[0], *z.shape[1:]), z.dtype)
            for z in self.zero_outs
        ]
        return concat

    def run(self, args):
        outs = self.fn(*args)
        return [
            {nm: np.asarray(outs[i]).reshape(NCORES, *self.out_avals[i].shape)[c]
             for i, nm in enumerate(self.out_names)}
            for c in range(NCORES)
        ]


# ----------------------------------------------------------------------------
# Host-side sharding / index plumbing
# ----------------------------------------------------------------------------

def _round_up(v, m):
    return (v + m - 1) // m * m


def _pow2_round(v):
    p = 1
    while p < v:
        p *= 2
    return p


def prepare(x, group_indices, pool_cluster_fine, batch_cluster_coarse,
            W1, b1, W2, b2, w_out, b_out):
    """Compute capacities + per-core input maps. Returns (key, in_maps, meta)."""
    bf16 = mybir.dt.np(BF16)
    x = np.asarray(x, dtype=np.float32)
    gidx = np.asarray(group_indices)
    pcf = np.asarray(pool_cluster_fine).astype(np.int64)
    bcc = np.asarray(batch_cluster_coarse).astype(np.int64)
    W1 = np.asarray(W1, dtype=np.float32)
    b1 = np.asarray(b1, dtype=np.float32)
    W2 = np.asarray(W2, dtype=np.float32)
    w_out = np.asarray(w_out, dtype=np.float32)
    b_out = np.asarray(b_out, dtype=np.float32)
    # b2 is provably irrelevant: per-channel constant shift before a
    # per-channel max and instance-norm standardization -> cancels exactly.

    GPC = G_SEG // NCORES

    # node -> group (later groups win on duplicates, matching scatter order)
    gid = np.full(N, -1, np.int32)
    for g in range(NG):
        gid[gidx[g]] = g

    # cluster boundaries for every fine cluster
    cl_lo = np.searchsorted(pcf, np.arange(F_SEG))
    cl_hi = np.searchsorted(pcf, np.arange(F_SEG), "right")
    cl_sz = cl_hi - cl_lo
    MCAP = _pow2_round(max(1, int(cl_sz.max())))

    # graph boundaries in cluster and node space
    g_lo = np.searchsorted(bcc, np.arange(G_SEG))
    g_hi = np.searchsorted(bcc, np.arange(G_SEG), "right")
    g_sz = g_hi - g_lo
    CCAP = _round_up(max(1, int(g_sz.max())), P)
    gn_lo = np.searchsorted(pcf, g_lo)      # first node of each graph
    gn_hi = np.searchsorted(pcf, g_hi)      # one past last node

    # per-graph per-expert row counts, then balance graphs across cores so
    # max over (core, expert) of the summed count -- the padded GEMM
    # capacity -- is minimized (greedy largest-first bin packing)
    gcnt = np.zeros((G_SEG, NG), np.int64)
    for gg in range(G_SEG):
        gs = gid[gn_lo[gg]:gn_hi[gg]]
        for g in range(NG):
            gcnt[gg, g] = int((gs == g).sum())
    order = np.argsort(-gcnt.max(axis=1), kind="stable")
    core_graphs = [[] for _ in range(NCORES)]
    loads = np.zeros((NCORES, NG), np.int64)
    for gg in order:
        best, best_val = -1, None
        for c in range(NCORES):
            if len(core_graphs[c]) >= GPC:
                continue
            val = int((loads[c] + gcnt[gg]).max())
            if best_val is None or val < best_val:
                best, best_val = c, val
        core_graphs[best].append(int(gg))
        loads[best] += gcnt[gg]
    core_graphs = [sorted(gl) for gl in core_graphs]

    counts = np.zeros((NCORES, NG), np.int64)
    core_nodes = []
    for c in range(NCORES):
        nd = np.concatenate([np.arange(gn_lo[gg], gn_hi[gg])
                             for gg in core_graphs[c]])
        core_nodes.append(nd)
        gs = gid[nd]
        for g in range(NG):
            counts[c, g] = int((gs == g).sum())
    GCAP = _round_up(max(1, int(counts.max())), 128)
    RTOT = NG * GCAP
    assert 2 + RTOT < 32768, f"GCAP={GCAP} too large for int16 gather indices"
    SLOTS = GPC * CCAP
    SUB = 32                       # finest gather-block granularity
    NSUB = SLOTS // SUB
    CHUNKS = _chunk_sizes(GCAP)
    NCS = len(CHUNKS)
    OFFS = np.concatenate([[0], np.cumsum(CHUNKS)]).astype(int)

    # replicated weight prep (shared across cores)
    w1_h = np.ascontiguousarray(W1.transpose(1, 0, 2)).astype(bf16)
    w2_h = np.ascontiguousarray(
        W2.reshape(NG, KEXP // P, P, H).transpose(2, 0, 1, 3)).astype(bf16)
    b1_h = np.ascontiguousarray(
        b1.reshape(NG, KEXP // P, P).transpose(2, 0, 1).reshape(P, -1))
    wo_h = np.ascontiguousarray(
        w_out.reshape(H // P, P, C_CLS).transpose(1, 0, 2)).astype(bf16)
    bo_h = np.ascontiguousarray(b_out.reshape(C_CLS, 1))

    in_maps = []
    meta = []
    ready_sub = np.zeros(NSUB, np.int64)
    members_c, invs_c = [], []
    for c in range(NCORES):
        nd = core_nodes[c]
        gs = gid[nd]
        xt = np.zeros((P, NG, GCAP), bf16)
        rows = np.zeros(N, np.int32)     # node -> og row (0 = zero row)
        for g in range(NG):
            sel = nd[gs == g]
            cnt = len(sel)
            xt[:, g, :cnt] = x[sel].T.astype(bf16)
            rows[sel] = 2 + g * GCAP + np.arange(cnt, dtype=np.int32)

        # member table: [SLOTS, MCAP] og-row indices.  Unused slots keep all
        # members = row 0 (zeros -> emb 0, sums unaffected); real clusters
        # pad their member tail with row 1 (-inf, neutral for max).
        member = np.zeros((SLOTS, MCAP), np.int32)
        inv_cnt = np.zeros(GPC, np.float32)
        for gi, gg in enumerate(core_graphs[c]):
            inv_cnt[gi] = 1.0 / max(int(g_sz[gg]), 1)
            for f in range(int(g_lo[gg]), int(g_hi[gg])):
                slot = gi * CCAP + (f - int(g_lo[gg]))
                sz = int(cl_sz[f])
                member[slot, :sz] = rows[int(cl_lo[f]) + np.arange(sz)]
                member[slot, sz:] = 1
        members_c.append(member)
        invs_c.append(inv_cnt)

        # readiness at SUB granularity: last chunk-set with any member row
        rank = (member - 2) % GCAP                        # rank within group
        rank[member < 2] = 0                              # sentinels: ready at 0
        cs_of_rank = np.searchsorted(OFFS[1:], rank, "right")
        ready_sub = np.maximum(
            ready_sub, cs_of_rank.reshape(NSUB, SUB * MCAP).max(axis=1))

        in_maps.append({
            "xt": xt,
            "w1": w1_h, "w2": w2_h, "b1s": b1_h,
            "wout": wo_h, "bout": bo_h,
            "invc": np.broadcast_to(inv_cnt[None, :], (P, GPC)).copy(),
        })
        meta.append({"graphs": core_graphs[c], "c": c})

    # gather-block plan: merge 4 aligned SUB blocks into one 128-cluster
    # block unless the merge would drag an earlier sub-block into the final
    # chunk-set (that would grow the post-GEMM tail)
    plan = []
    for t0 in range(0, NSUB, 4):
        rds = ready_sub[t0:t0 + 4]
        if rds.max() == NCS - 1 and rds.min() < NCS - 1:
            for k in range(4):
                plan.append((int((t0 + k) * SUB), SUB, int(rds[k])))
        else:
            plan.append((int(t0 * SUB), 4 * SUB, int(rds.max())))
    plan = tuple(plan)

    # concatenated per-plan-item dma_gather index tables
    gi_cols = sum(MCAP * ncl // 16 for (c0, ncl, rdy) in plan)
    for c in range(NCORES):
        member = members_c[c]
        gidx_w = np.zeros((P, gi_cols), np.int16)
        o = 0
        for (c0, ncl, rdy) in plan:
            nidx = MCAP * ncl
            mt = member[c0:c0 + ncl]                      # [ncl, MCAP]
            seq = mt.T.reshape(-1)                        # i = m*ncl + c
            w = seq.reshape(-1, 16).T.astype(np.int16)    # [16, nidx/16]
            gidx_w[:, o:o + nidx // 16] = np.tile(w, (8, 1))
            o += nidx // 16
        in_maps[c]["gidx"] = gidx_w

    key = (GCAP, CCAP, MCAP, plan)
    return key, in_maps, meta, (CCAP,)


def get_runner(key, phases=5, repeat=1):
    ck = (key, phases, repeat)
    if ck not in _PROGRAM_CACHE:
        GCAP, CCAP, MCAP, plan = key
        nc = _build_program(GCAP, CCAP, MCAP, plan, phases=phases,
                            repeat=repeat)
        _PROGRAM_CACHE[ck] = _Runner(nc)
    return _PROGRAM_CACHE[ck]


def kernel(**inputs) -> np.ndarray:
    key, in_maps, meta, (CCAP,) = prepare(**inputs)
    runner = get_runner(key)
    args = runner.prepare(in_maps)
    results = runner.run(args)

    bcc = np.asarray(inputs["batch_cluster_coarse"]).astype(np.int64)
    g_lo = np.searchsorted(bcc, np.arange(G_SEG))
    g_hi = np.searchsorted(bcc, np.arange(G_SEG), "right")
    out = np.zeros((F_SEG, C_CLS), np.float32)
    for c in range(NCORES):
        lo = results[c]["logt"]              # [16, SLOTS]
        for gi, gg in enumerate(meta[c]["graphs"]):
            fs = np.arange(int(g_lo[gg]), int(g_hi[gg]))
            out[fs] = lo[:, gi * CCAP + (fs - int(g_lo[gg]))].T
    return out
